# revision 69
# baseline (speedup 1.0000x reference)
"""Trainium2 Bass kernel: AdaptivePointNet2 feature propagation (KNN k=3 +
inverse-distance interpolation + 2x [conv1x1 -> BN(train) -> ReLU]).

Sharding: one frame per NeuronCore (8 frames, 8 cores). point2frameidx /
query2frameidx are sorted, so each frame's points/queries are contiguous
slices; the per-frame KNN mask then becomes "no mask" on-device. Padded to
fixed caps; BatchNorm statistics are all-reduced across cores.

Device pipeline (per core, NP points, MQ queries):
  A. s = -d2 via one K=33 matmul per 128-point chunk (exact 3-piece bf16
     splits reproduce the fp32 gram-trick values to ~1 ulp so top-k
     selection matches the reference).
  B. top-8 values+indices per chunk (DVE max8 / max-index), inverse-distance
     weights, validity-masked.
  C. per group of NCH/3 chunks: indices+weights rearranged through a DRAM
     bounce into the gpsimd wrapped-index layout (one packed write, one
     stride-0 broadcast read-back); ap_gather pulls interleaved bf16
     feature pairs; PE broadcasts the weights across partitions; DVE does
     the weighting and k-sum in bf16 fast mode.
  D. MLP layer: bf16 matmuls into PSUM, ACT evacuates, DVE bn_stats;
     cross-core BN stats via a tiny AllGather + on-device reduction
     (AllGather floor ~4.6us vs AllReduce ~9.7us on trn2); BN apply +
     ReLU fused in one ACT op (h0) / gpsimd tensor_scalar pair (h1).
"""

import numpy as np
from functools import lru_cache

N_CORES = 8
N_TOT = 16384  # total points (BN divisor)
DIST_EPS = 1e-8
BN_EPS = 1e-5
PAD_COORD = 1.0e4  # padded query coordinate -> enormous distance, never selected
USE_COLLECTIVE = True
GROUPS = 3


# ---------------------------------------------------------------------------
# device program
# ---------------------------------------------------------------------------
@lru_cache(maxsize=4)
def _build_bass(NP: int, MQ: int):
    import concourse.tile as tile
    import concourse.bass as bass
    from concourse import bacc, mybir

    f32 = mybir.dt.float32
    bf16 = mybir.dt.bfloat16
    u16 = mybir.dt.uint16
    i16 = mybir.dt.int16
    AF = mybir.ActivationFunctionType
    ALU = mybir.AluOpType
    AX = mybir.AxisListType

    NCH = NP // 128              # point chunks of 128
    NCHG = NCH // GROUPS         # chunks per gather group
    NPG = NCHG * 128             # points per group
    JTG = 3 * NPG                # gathered items per group
    KA = 33                      # bf16-split rows of the -d2 matmul
    INV_N = 1.0 / float(N_TOT)
    FNP = float(NP)

    def csplit(total, step):
        return [(o, min(step, total - o)) for o in range(0, total, step)]

    MQ_SPLIT = csplit(MQ, 512)
    JTG_SPLIT = csplit(JTG, 512)
    COL_SPLIT = [(g * NPG + o, sz) for g in range(GROUPS)
                 for o, sz in csplit(NPG, 384)]

    nc = bacc.Bacc(None, target_bir_lowering=False, debug=False)

    xs = nc.declare_dram_parameter("xs", [KA, NP], bf16, isOutput=False)
    yq = nc.declare_dram_parameter("yq", [KA, MQ], bf16, isOutput=False)
    feat = nc.declare_dram_parameter("feat", [128, NP], bf16, isOutput=False)
    fp = nc.declare_dram_parameter("fp", [128, MQ, 2], bf16, isOutput=False)
    v18 = nc.declare_dram_parameter("v18", [128, NCH], f32, isOutput=False)
    vcb = nc.declare_dram_parameter("vcb", [128, NP], bf16, isOutput=False)
    w1t = nc.declare_dram_parameter("w1t", [384, 256], bf16, isOutput=False)
    w2t = nc.declare_dram_parameter("w2t", [256, 256], bf16, isOutput=False)
    bnp = nc.declare_dram_parameter("bnp", [256, 4], f32, isOutput=False)
    out = nc.declare_dram_parameter("out", [256, NP], bf16, isOutput=True)

    with tile.TileContext(nc) as tc:
        from contextlib import ExitStack

        with ExitStack() as ctx:
            const = ctx.enter_context(tc.tile_pool(name="const", bufs=1))
            work = ctx.enter_context(tc.tile_pool(name="work", bufs=4))
            big = ctx.enter_context(tc.tile_pool(name="big", bufs=1))
            gpool = ctx.enter_context(tc.tile_pool(name="gpool", bufs=2))
            psA = ctx.enter_context(tc.tile_pool(name="psA", bufs=2, space="PSUM"))
            psB = ctx.enter_context(tc.tile_pool(name="psB", bufs=2, space="PSUM"))
            dram = ctx.enter_context(tc.tile_pool(name="dram", bufs=1, space="DRAM"))

            # ------------------------------------------------ constant loads
            yq_t = const.tile([KA, MQ], bf16)
            nc.sync.dma_start(out=yq_t, in_=yq[:])
            xs_t = const.tile([KA, NP], bf16)
            for g in range(GROUPS):
                nc.sync.dma_start(
                    out=xs_t[:, g * NPG : (g + 1) * NPG],
                    in_=xs[:, g * NPG : (g + 1) * NPG],
                )
            feat_t = const.tile([128, NP], bf16)
            nc.gpsimd.dma_start(out=feat_t, in_=feat[:])
            fp_t = const.tile([128, MQ, 2], bf16)
            nc.gpsimd.dma_start(out=fp_t, in_=fp[:])
            v18_t = const.tile([128, NCH], f32)
            nc.sync.dma_start(out=v18_t, in_=v18[:])
            vcb_t = const.tile([128, NP], bf16)
            nc.gpsimd.dma_start(out=vcb_t, in_=vcb[:])
            w1t_t = const.tile([128, 3, 256], bf16)
            nc.gpsimd.dma_start(out=w1t_t, in_=w1t[:].rearrange("(k p) d -> p k d", k=3))
            w2t_t = const.tile([128, 2, 256], bf16)
            nc.gpsimd.dma_start(out=w2t_t, in_=w2t[:].rearrange("(k p) d -> p k d", k=2))
            bnp_t = const.tile([128, 2, 4], f32)
            nc.sync.dma_start(out=bnp_t, in_=bnp[:].rearrange("(h p) s -> p h s", h=2))
            g1_t = bnp_t[:, :, 0]
            be1_t = bnp_t[:, :, 1]
            g2_t = bnp_t[:, :, 2]
            be2_t = bnp_t[:, :, 3]
            ones_t = const.tile([1, 128], bf16)
            nc.vector.memset(ones_t, 1.0)
            eps_t = const.tile([128, 1], f32)
            nc.vector.memset(eps_t, BN_EPS)

            vmax_t = big.tile([128, NCH, 8], f32)
            vidx_t = big.tile([128, NCH, 8], u16)
            wbf_t = big.tile([128, NCH * 3], bf16)
            idx_dr = [dram.tile([128, NCHG * 3], i16, tag=f"idx_dr{g}",
                                name=f"idx_dr{g}") for g in range(GROUPS)]
            wbf_dr = [dram.tile([128, NCHG * 3], bf16, tag=f"wbf_dr{g}",
                                name=f"wbf_dr{g}") for g in range(GROUPS)]
            interp_t = big.tile([128, NP, 2], bf16, tag="interp", name="interp")

            def knn_chunk(ic):
                """-d2 matmul + top-8 + weights for point chunk ic."""
                s_ps = psA.tile([128, MQ], f32, tag="s_ps")
                for off, sz in MQ_SPLIT:
                    nc.tensor.matmul(
                        out=s_ps[:, off : off + sz],
                        lhsT=xs_t[:, ic * 128 : (ic + 1) * 128],
                        rhs=yq_t[:, off : off + sz],
                        start=True,
                        stop=True,
                    )
                s_sb = work.tile([128, MQ], f32, tag="s_sb")
                nc.scalar.activation(out=s_sb, in_=s_ps, func=AF.Copy)
                nc.vector.max(out=vmax_t[:, ic, :], in_=s_sb)
                nc.vector.max_index(
                    out=vidx_t[:, ic, :], in_max=vmax_t[:, ic, :], in_values=s_sb
                )

            def gather_group(g):
                """idx/weight rearrange + gather + weighting + k-sum, group g."""
                r0 = g * NCHG * 3
                c0, c1 = g * NCHG, (g + 1) * NCHG
                # batched weights: w = (1/(d+eps)) / sum_k * valid
                w3 = work.tile([128, NCHG, 3], f32, tag="w3", name="w3")
                nc.vector.tensor_scalar_min(w3, vmax_t[:, c0:c1, 0:3], 0.0)
                nc.scalar.activation(out=w3, in_=w3, func=AF.Sqrt, scale=-1.0)
                wi = work.tile([128, NCHG, 3], f32, tag="wi", name="wi")
                nc.vector.tensor_scalar_add(wi, w3, DIST_EPS)
                nc.vector.reciprocal(out=wi, in_=wi)
                ws = work.tile([128, NCHG], f32, tag="ws", name="ws")
                nc.vector.tensor_reduce(out=ws, in_=wi, axis=AX.X, op=ALU.add)
                nc.vector.reciprocal(out=ws, in_=ws)
                nc.vector.tensor_mul(ws, ws, v18_t[:, c0:c1])
                nc.vector.tensor_mul(
                    wbf_t[:, r0 : r0 + NCHG * 3].rearrange(
                        "p (a b) -> p a b", b=3
                    ),
                    wi,
                    ws[:].to_broadcast((128, NCHG, 3)),
                )
                nc.sync.dma_start(
                    out=idx_dr[g],
                    in_=vidx_t[:, g * NCHG : (g + 1) * NCHG, 0:3].bitcast(i16),
                )
                nc.sync.dma_start(
                    out=wbf_dr[g],
                    in_=wbf_t[:, r0 : r0 + NCHG * 3],
                )
                idxg = gpool.tile([128, JTG // 16], i16, tag="idxg", name="idxg")
                idflat = idx_dr[g][:].rearrange("(a b) r -> a (b r)", a=16)
                rep = bass.AP(
                    tensor=idflat.tensor,
                    offset=idflat.offset,
                    ap=[[0, 8]] + list(idflat.ap),
                )
                nc.sync.dma_start(out=idxg, in_=rep)
                wrow = gpool.tile([1, JTG], bf16, tag="wrow", name="wrow")
                nc.sync.dma_start(
                    out=wrow,
                    in_=wbf_dr[g][:].rearrange("(a b) r -> b r a", a=16),
                )
                G_t = gpool.tile([128, JTG, 2], bf16, tag="G", name="G")
                JH = JTG // 2
                for jh in range(2):
                    nc.gpsimd.ap_gather(
                        out_ap=G_t[:, jh * JH : (jh + 1) * JH, :],
                        in_ap=fp_t,
                        idxs_ap=idxg[:, jh * (JH // 16) : (jh + 1) * (JH // 16)],
                        channels=128,
                        num_elems=MQ,
                        d=2,
                        num_idxs=JH,
                    )
                wb2 = gpool.tile([128, JTG, 2], bf16, tag="wb2", name="wb2")
                for off, sz in JTG_SPLIT:
                    wb_ps = psB.tile([128, 512], f32, tag="wb")
                    nc.tensor.matmul(
                        out=wb_ps[:, :sz],
                        lhsT=ones_t,
                        rhs=wrow[:, off : off + sz],
                        start=True,
                        stop=True,
                    )
                    for h in range(2):
                        nc.scalar.activation(
                            out=wb2[:, off : off + sz, h], in_=wb_ps[:, :sz],
                            func=AF.Copy,
                        )
                    nc.vector.tensor_mul(
                        G_t[:, off : off + sz, :], G_t[:, off : off + sz, :],
                        wb2[:, off : off + sz, :],
                    )
                Gv = G_t[:].rearrange(
                    "p (q three t) h -> p q three t h", three=3, t=16
                )
                kt = gpool.tile([128, NPG, 2], bf16, tag="ktmp", name="ktmp")
                kv = kt[:].rearrange("p (q t) h -> p q t h", t=16)
                iv = interp_t[:, g * NPG : (g + 1) * NPG, :].rearrange(
                    "p (q t) h -> p q t h", t=16
                )
                nc.vector.tensor_add(kv, Gv[:, :, 0, :, :], Gv[:, :, 1, :, :])
                nc.vector.tensor_add(iv, kv, Gv[:, :, 2, :, :])


            def mlp_layer(xfn, wt_t, nkc, pref, lhs_map=None):
                """y = W @ x (bf16 matmuls, f32 PSUM), ACT-evac to SBUF,
                per-chunk DVE bn_stats -> (mean, var) -> local sums.
                Returns (y_t, st_t [128, 4] = (sum_h0, sum_h1, sq_h0, sq_h1))."""
                y_t = big.tile([128, 2, NP], f32, tag=f"{pref}_y", name=f"{pref}_y")
                nchunks = len(COL_SPLIT)
                st_t = big.tile([128, 4], f32, tag=f"{pref}_st", name=f"{pref}_st")
                bst_t = big.tile([128, 2, nchunks, 6], f32, tag=f"{pref}_bst",
                                 name=f"{pref}_bst")
                for ci, (off, sz) in enumerate(COL_SPLIT):
                    for h in range(2):
                        y_ps = psB.tile([128, 384], f32, tag="y")
                        for kc in range(nkc):
                            kw = lhs_map(kc) if lhs_map is not None else kc
                            nc.tensor.matmul(
                                out=y_ps[:, :sz],
                                lhsT=wt_t[:, kw, h * 128 : (h + 1) * 128],
                                rhs=xfn(kc, off, sz),
                                start=(kc == 0),
                                stop=(kc == nkc - 1),
                            )
                        nc.scalar.activation(
                            out=y_t[:, h, off : off + sz], in_=y_ps[:, :sz],
                            func=AF.Copy,
                        )
                        nc.vector.bn_stats(
                            out=bst_t[:, h, ci, :],
                            in_=y_t[:, h, off : off + sz],
                        )
                mv_t = big.tile([128, 2, 2], f32, tag=f"{pref}_mv",
                                name=f"{pref}_mv")
                for h in range(2):
                    nc.vector.bn_aggr(out=mv_t[:, h, :], in_=bst_t[:, h, :, :])
                nc.vector.tensor_scalar_mul(st_t[:, 0:2], mv_t[:, :, 0], FNP)
                m2 = work.tile([128, 2], f32, tag="m2", name="m2")
                nc.vector.tensor_mul(m2, mv_t[:, :, 0], mv_t[:, :, 0])
                sq = work.tile([128, 2], f32, tag="sq", name="sq")
                nc.vector.tensor_add(sq, mv_t[:, :, 1], m2)
                nc.vector.tensor_scalar_mul(st_t[:, 2:4], sq, FNP)
                return y_t, st_t

            def bn_coefs(st_t, gref, beref, tagp):
                """local (sum,sumsq) -> AllGather + reduce -> a,b."""
                gst_t = big.tile([128, 4], f32, tag=f"{tagp}_gst", name=f"{tagp}_gst")
                if USE_COLLECTIVE:
                    ar_in = dram.tile([128, 4], f32, tag=f"{tagp}_ar_in",
                                      name=f"{tagp}_ar_in")
                    ar_out = dram.tile([128 * N_CORES, 4], f32, tag=f"{tagp}_ar_out",
                                       name=f"{tagp}_ar_out")
                    nc.sync.dma_start(out=ar_in, in_=st_t)
                    nc.gpsimd.collective_compute(
                        "AllGather",
                        ALU.bypass,
                        replica_groups=[list(range(N_CORES))],
                        ins=[ar_in.opt()],
                        outs=[ar_out.opt()],
                    )
                    ag_t = big.tile([128, 4, N_CORES], f32, tag=f"{tagp}_ag",
                                    name=f"{tagp}_ag")
                    nc.sync.dma_start(
                        out=ag_t, in_=ar_out[:].rearrange("(r p) s -> p s r", r=N_CORES)
                    )
                    nc.vector.tensor_reduce(
                        out=gst_t, in_=ag_t, axis=AX.X, op=ALU.add
                    )
                else:
                    nc.vector.tensor_scalar_mul(gst_t, st_t, float(N_CORES))
                mean_t = big.tile([128, 2], f32, tag=f"{tagp}_mean",
                                  name=f"{tagp}_mean")
                nc.vector.tensor_scalar_mul(mean_t, gst_t[:, 0:2], INV_N)
                m2_t = big.tile([128, 2], f32, tag=f"{tagp}_m2", name=f"{tagp}_m2")
                nc.vector.tensor_mul(m2_t, mean_t, mean_t)
                var_t = big.tile([128, 2], f32, tag=f"{tagp}_var", name=f"{tagp}_var")
                nc.vector.scalar_tensor_tensor(
                    out=var_t, in0=gst_t[:, 2:4], scalar=INV_N, in1=m2_t,
                    op0=ALU.mult, op1=ALU.subtract,
                )
                a_t = big.tile([128, 2], f32, tag=f"{tagp}_a", name=f"{tagp}_a")
                nc.scalar.activation(out=a_t, in_=var_t, func=AF.Sqrt, bias=eps_t)
                nc.vector.reciprocal(out=a_t, in_=a_t)
                nc.vector.tensor_mul(a_t, a_t, gref)
                b_t = big.tile([128, 2], f32, tag=f"{tagp}_b", name=f"{tagp}_b")
                nc.vector.tensor_mul(b_t, mean_t, a_t)
                nc.vector.tensor_sub(b_t, beref, b_t)
                return a_t, b_t

            # ---------------------------- pipeline: knn chunks + gather groups
            for g in range(GROUPS):
                for ic in range(g * NCHG, (g + 1) * NCHG):
                    knn_chunk(ic)
                gather_group(g)

            # ------------------------------------------- layer 1 + BN1 + relu
            def x1_rhs(kc, off, sz):
                if kc < 2:
                    return interp_t[:, off : off + sz, kc]
                return feat_t[:, off : off + sz]
            y1_t, st1_t = mlp_layer(x1_rhs, w1t_t, 3, "l1")
            a1_t, b1_t = bn_coefs(st1_t, g1_t, be1_t, "bn1")
            x2_t = [
                big.tile([128, NP], bf16, tag="x20", name="x20"),
                big.tile([128, NP], bf16, tag="x21", name="x21"),
            ]
            for off, sz in COL_SPLIT:
                for h in range(2):
                    if h == 0:
                        nc.scalar.activation(
                            out=x2_t[h][:, off : off + sz],
                            in_=y1_t[:, h, off : off + sz],
                            func=AF.Relu,
                            scale=a1_t[:, h : h + 1],
                            bias=b1_t[:, h : h + 1],
                        )
                    else:
                        nc.gpsimd.tensor_scalar(
                            out=x2_t[h][:, off : off + sz],
                            in0=y1_t[:, h, off : off + sz],
                            scalar1=a1_t[:, h : h + 1],
                            scalar2=b1_t[:, h : h + 1],
                            op0=ALU.mult,
                            op1=ALU.add,
                        )
                        nc.gpsimd.tensor_scalar_max(
                            x2_t[h][:, off : off + sz],
                            x2_t[h][:, off : off + sz], 0.0,
                        )
                    nc.vector.tensor_mul(
                        x2_t[h][:, off : off + sz], x2_t[h][:, off : off + sz],
                        vcb_t[:, off : off + sz],
                    )

            # ------------------------------------------- layer 2 + BN2 + relu
            def x2_rhs(kc, off, sz):
                return x2_t[kc][:, off : off + sz]
            y2_t, st2_t = mlp_layer(x2_rhs, w2t_t, 2, "l2")
            a2_t, b2_t = bn_coefs(st2_t, g2_t, be2_t, "bn2")
            out2_t = big.tile([128, 2, NP], bf16, tag="out2", name="out2")
            outv = out[:].rearrange("(h p) n -> p h n", h=2)
            for off, sz in COL_SPLIT:
                for h in range(2):
                    if h == 0:
                        nc.scalar.activation(
                            out=out2_t[:, h, off : off + sz],
                            in_=y2_t[:, h, off : off + sz],
                            func=AF.Relu,
                            scale=a2_t[:, h : h + 1],
                            bias=b2_t[:, h : h + 1],
                        )
                    else:
                        nc.gpsimd.tensor_scalar(
                            out=out2_t[:, h, off : off + sz],
                            in0=y2_t[:, h, off : off + sz],
                            scalar1=a2_t[:, h : h + 1],
                            scalar2=b2_t[:, h : h + 1],
                            op0=ALU.mult,
                            op1=ALU.add,
                        )
                        nc.gpsimd.tensor_scalar_max(
                            out2_t[:, h, off : off + sz],
                            out2_t[:, h, off : off + sz], 0.0,
                        )
                nc.sync.dma_start(
                    out=outv[:, :, off : off + sz],
                    in_=out2_t[:, :, off : off + sz],
                )

    nc.finalize()
    return nc


# ---------------------------------------------------------------------------
# host-side sharding helpers
# ---------------------------------------------------------------------------
def _caps(n_sizes, m_sizes):
    NP = max(1152, int(-(-max(n_sizes) // 384)) * 384)
    MQ = max(64, int(-(-max(m_sizes) // 16)) * 16)
    return NP, MQ


def _perm(NP):
    """Device interp-column order c -> natural point index n (within shard).

    Within each gather group g (NCHG chunks of 128 points):
      c_local = (NCHG*p0 + nc_local)*16 + p16 for point
      n_local = nc_local*128 + (8*p16 + p0);  c = g*NPG + c_local.
    """
    NCH = NP // 128
    NCHG = NCH // GROUPS
    NPG = NCHG * 128
    c = np.arange(NP)
    g = c // NPG
    cl = c % NPG
    p16 = cl % 16
    ql = cl // 16
    p0 = ql // NCHG
    ncl = ql % NCHG
    return (g * NCHG + ncl) * 128 + 8 * p16 + p0


def _split3(v):
    """Exact 3-piece bf16 split: hi+mid+lo == v to ~2^-25 relative."""
    import ml_dtypes
    bf = ml_dtypes.bfloat16
    hi = v.astype(bf).astype(np.float32)
    r = (v - hi).astype(np.float32)
    mid = r.astype(bf).astype(np.float32)
    lo = (r - mid).astype(bf).astype(np.float32)
    return hi, mid, lo


def _aug_rows(X, Y2, sqx, sqy):
    """K=33 bf16 operand rows for s = 2x.y - |x|^2 - |y|^2.
    X [N,3], Y2 [M,3] (=2*xyz_prev), sqx [N], sqy [M] -> (xs [33,N], yq [33,M])."""
    import ml_dtypes
    bf = ml_dtypes.bfloat16
    N, M = X.shape[0], Y2.shape[0]
    xp = [_split3(X[:, d]) for d in range(3)]
    yp = [_split3(Y2[:, d]) for d in range(3)]
    sxp = _split3(sqx)
    syp = _split3(sqy)
    xs = np.zeros((33, N), np.float32)
    yq = np.zeros((33, M), np.float32)
    r = 0
    for d in range(3):
        for px in range(3):
            for py in range(3):
                xs[r] = xp[d][px]
                yq[r] = yp[d][py]
                r += 1
    for p in range(3):
        xs[27 + p] = sxp[p]
        yq[27 + p] = -1.0
        xs[30 + p] = 1.0
        yq[30 + p] = -syp[p]
    return xs.astype(bf), yq.astype(bf)


def _shard_inputs(xyz, xyz_prev, features, features_prev, p2f, q2f,
                  W1, g1, be1, W2, g2, be2):
    import ml_dtypes
    bf = ml_dtypes.bfloat16
    pb = np.searchsorted(p2f, np.arange(N_CORES + 1))
    qb = np.searchsorted(q2f, np.arange(N_CORES + 1))
    n_sizes = np.diff(pb)
    m_sizes = np.diff(qb)
    NP, MQ = _caps(n_sizes, m_sizes)
    NCH = NP // 128
    n_of_c = _perm(NP)

    w1t = np.ascontiguousarray(W1.T).astype(bf)
    w2t = np.ascontiguousarray(W2.T).astype(bf)
    bnp = np.ascontiguousarray(
        np.stack([g1, be1, g2, be2], axis=1).astype(np.float32)
    )

    in_maps = []
    metas = []
    for f in range(N_CORES):
        ns, ne = int(pb[f]), int(pb[f + 1])
        ms, me = int(qb[f]), int(qb[f + 1])
        nf, mf = ne - ns, me - ms
        X = np.zeros((NP, 3), np.float32)
        X[:nf] = xyz[ns:ne]
        sqx = (X * X).sum(1)
        Y = np.full((MQ, 3), PAD_COORD, np.float32)
        Y[:mf] = xyz_prev[ms:me]
        sqy = (Y * Y).sum(1)
        xsr, yqr = _aug_rows(X, (2.0 * Y).astype(np.float32), sqx, sqy)
        F = np.zeros((128, NP), np.float32)
        F[:, :nf] = features[:, ns:ne]
        Fc = np.ascontiguousarray(F[:, n_of_c]).astype(bf)
        FP = np.zeros((256, MQ), np.float32)
        FP[:, :mf] = features_prev[:, ms:me]
        FP = np.ascontiguousarray(np.stack([FP[:128], FP[128:]], axis=-1)).astype(bf)
        valid_n = (np.arange(NP) < nf)
        v18a = np.ascontiguousarray(valid_n.reshape(NCH, 128).T.astype(np.float32))
        vc = valid_n[n_of_c].astype(np.float32)
        vcb = np.ascontiguousarray(np.broadcast_to(vc, (128, NP))).astype(bf)
        in_maps.append(
            dict(xs=xsr, yq=yqr, feat=Fc, fp=FP, v18=v18a, vcb=vcb,
                 w1t=w1t, w2t=w2t, bnp=bnp)
        )
        metas.append((ns, nf))
    return NP, MQ, n_of_c, in_maps, metas


def _unshard(results, metas, n_of_c, out_dtype=np.float32):
    out = np.empty((256, N_TOT), out_dtype)
    for f, (ns, nf) in enumerate(metas):
        dev = np.asarray(results[f]["out"], dtype=out_dtype)
        sel = n_of_c < nf
        out[:, ns + n_of_c[sel]] = dev[:, sel]
    return out


def kernel(xyz, xyz_prev, features, features_prev, point2frameidx, query2frameidx,
           W1, b1, g1, be1, W2, b2, g2, be2):
    # b1/b2 cancel inside the training-mode BatchNorm (constant shift along the
    # normalized axis), so they are accepted but unused.
    from concourse.bass_utils import run_bass_kernel_spmd

    xyz = np.asarray(xyz, np.float32)
    xyz_prev = np.asarray(xyz_prev, np.float32)
    features = np.asarray(features, np.float32)
    features_prev = np.asarray(features_prev, np.float32)
    p2f = np.asarray(point2frameidx)
    q2f = np.asarray(query2frameidx)

    NP, MQ, n_of_c, in_maps, metas = _shard_inputs(
        xyz, xyz_prev, features, features_prev, p2f, q2f,
        np.asarray(W1, np.float32), np.asarray(g1, np.float32),
        np.asarray(be1, np.float32), np.asarray(W2, np.float32),
        np.asarray(g2, np.float32), np.asarray(be2, np.float32),
    )
    nc = _build_bass(NP, MQ)
    res = run_bass_kernel_spmd(nc, in_maps, list(range(N_CORES)))
    return _unshard(res.results, metas, n_of_c)
